# revision 1
# baseline (speedup 1.0000x reference)
"""GroupedRouter Bass kernel for 8 TRN2 NeuronCores.

Reference computation (per batch b, head h):
    q = x @ Wq, k = x @ Wk           (heads of dim 128)
    scores = q k^T / sqrt(128)       [N, N]
    group max over 8 key groups of 128, keep top-2 groups, softmax over kept.

Sharding: core c -> batch b = c//2, head half hh = c%2 (8 heads per core).
Each core computes out[b, :, hh*8:(hh+1)*8, :] locally: fully data-parallel,
no collectives.

Precision strategy: all matmuls run at bf16 rate using an error-compensated
bf16x2 split (v = v1 + v2 with v1 = bf16(v), v2 = bf16(v - v1)); products
keep ~2^-16 relative accuracy via three accumulating passes
(a1*b1 + a1*b2 + a2*b1) into fp32 PSUM. x and W are split host-side (same
total bytes as fp32); x is also transposed host-side into D-major layout, so
the kernel needs no on-chip transpose. q/k are re-split on-chip at the
PSUM->SBUF copyback.

Per-core pipeline:
  1) per head: stream Wq/Wk head slices (bf16 pair), 3-pass matmul ->
     qT,kT [128(dh), 1024(tok)] bf16 pairs (q scaled by 1/sqrt(128)).
  2) per (head, 128-query chunk): 3-pass scores -> PSUM [128,1024] fp32;
     grouped max (DVE reduce over [128,8,128]); top-2 threshold; per-group
     bias = -rowmax (kept) / -BIG (masked); ACT exp with bias + accumulated
     row-sum; reciprocal; GPSIMD normalize; DMA out.
"""
import numpy as np
import orjson
import ml_dtypes

import concourse.bass as bass
import concourse.mybir as mybir
from concourse.tile import TileContext
from concourse.bass_utils import run_bass_kernel_spmd
from concourse.bass import ts, ds

B, N, D = 4, 1024, 2048
H, DH = 16, 128
G = 8
GSIZE = N // G          # 128
NCORES = 8
HPC = H // 2            # heads per core
SCALE = float(1.0 / np.sqrt(DH))
BIG = 30000.0

f32 = mybir.dt.float32
bf16 = mybir.dt.bfloat16
Alu = mybir.AluOpType
Act = mybir.ActivationFunctionType
AxX = mybir.AxisListType.X

# ---------------------------------------------------------------------------
# BIR sync-wait legalizer: walrus for cayman accepts only one sync-wait
# command per instruction; Tile attaches one per dependency. Hoist the excess
# onto standalone EventSemaphore instructions immediately before the target
# (engine queues are FIFO, so blocking semantics are unchanged).
# ---------------------------------------------------------------------------

def _legalize_bir(bir: dict) -> dict:
    ctr = 0
    for fn in bir["functions"]:
        for bb in fn["blocks"]:
            insts = bb.get("instructions")
            if not insts:
                continue
            out = []
            for ins in insts:
                si = ins.get("sync_info")
                waits = (si or {}).get("on_wait") or []
                if len(waits) > 1:
                    for w in waits[:-1]:
                        ctr += 1
                        out.append({
                            "engine": ins["engine"],
                            "ins": [],
                            "outs": [],
                            "name": f"legwait-{ctr}",
                            "opcode": "EventSemaphore",
                            "sync_info": {"on_update": [], "on_wait": [w]},
                        })
                    si["on_wait"] = waits[-1:]
                out.append(ins)
            bb["instructions"] = out
    return bir


def _install_legalizer(nc):
    orig = nc.to_json_bytes

    def to_json_bytes():
        return orjson.dumps(_legalize_bir(orjson.loads(orig())))

    nc.to_json_bytes = to_json_bytes


# ---------------------------------------------------------------------------
# Kernel build (one SPMD program; per-core differences live in the input data)
# ---------------------------------------------------------------------------

def _build():
    nc = bass.Bass()
    # x[b] transposed host-side to D-major, split into bf16 hi/lo planes.
    xt1 = nc.declare_dram_parameter("xt1", [D, N], bf16, isOutput=False)
    xt2 = nc.declare_dram_parameter("xt2", [D, N], bf16, isOutput=False)
    wq1 = nc.declare_dram_parameter("wq1", [D, HPC * DH], bf16, isOutput=False)
    wq2 = nc.declare_dram_parameter("wq2", [D, HPC * DH], bf16, isOutput=False)
    wk1 = nc.declare_dram_parameter("wk1", [D, HPC * DH], bf16, isOutput=False)
    wk2 = nc.declare_dram_parameter("wk2", [D, HPC * DH], bf16, isOutput=False)
    out = nc.declare_dram_parameter("out", [N, HPC * N], f32, isOutput=True)

    nk = D // 128  # 16 contraction chunks
    xt1_3 = xt1.rearrange("(kc p) t -> p kc t", p=128)
    xt2_3 = xt2.rearrange("(kc p) t -> p kc t", p=128)
    w3 = [w.rearrange("(kc p) hd -> p kc hd", p=128)
          for w in (wq1, wq2, wk1, wk2)]

    with TileContext(nc) as tc:
        with tc.tile_pool(name="const", bufs=1) as cpool, \
             tc.tile_pool(name="xT", bufs=1) as xtp:
            negbig = cpool.tile([128, G], f32)
            nc.vector.memset(negbig[:], -BIG)

            # resident x planes: [128, kc*tok] bf16, 32KB/partition each
            xa = xtp.tile([128, nk * N], bf16, name="xa", tag="xa")
            xb_ = xtp.tile([128, nk * N], bf16, name="xb", tag="xb")
            nc.sync.dma_start(
                out=xa[:].rearrange("p (kc t) -> p kc t", t=N), in_=xt1_3[:])
            nc.sync.dma_start(
                out=xb_[:].rearrange("p (kc t) -> p kc t", t=N), in_=xt2_3[:])

            def xA(kc):
                return xa[:, ds(kc * N, N)]

            def xB(kc):
                return xb_[:, ds(kc * N, N)]

            with tc.tile_pool(name="w", bufs=2) as wpool, \
                 tc.tile_pool(name="qk", bufs=2) as qkp, \
                 tc.tile_pool(name="psp", bufs=1, space="PSUM") as psp, \
                 tc.tile_pool(name="pss", bufs=3, space="PSUM") as pss, \
                 tc.tile_pool(name="ep", bufs=4) as ep, \
                 tc.tile_pool(name="outp", bufs=3) as outp:
                for h in range(HPC):
                    # --- projections: 3-pass bf16x2 ---
                    qk_pair = []
                    for wi, (whi3, wlo3) in enumerate(
                            ((w3[0], w3[1]), (w3[2], w3[3]))):
                        whi = wpool.tile([128, nk * 128], bf16, tag="whi")
                        wlo = wpool.tile([128, nk * 128], bf16, tag="wlo")
                        nc.sync.dma_start(
                            out=whi[:].rearrange("p (kc hd) -> p kc hd", hd=128),
                            in_=whi3[:, :, ts(h, 128)])
                        nc.sync.dma_start(
                            out=wlo[:].rearrange("p (kc hd) -> p kc hd", hd=128),
                            in_=wlo3[:, :, ts(h, 128)])
                        ps = psp.tile([128, N], f32, tag="pp")
                        for half in range(2):
                            sl = ds(half * 512, 512)
                            passes = [(whi, xA), (wlo, xA), (whi, xB)]
                            for pi, (wt, xf) in enumerate(passes):
                                for kc in range(nk):
                                    nc.tensor.matmul(
                                        ps[:, sl], wt[:, ts(kc, 128)],
                                        xf(kc)[:, sl],
                                        start=(pi == 0 and kc == 0),
                                        stop=(pi == 2 and kc == nk - 1))
                        # copyback with bf16x2 re-split (scale q by 1/sqrt(dh))
                        s = SCALE if wi == 0 else 1.0
                        hi = qkp.tile([128, N], bf16, tag=f"hi{wi}")
                        lo = qkp.tile([128, N], bf16, tag=f"lo{wi}")
                        nc.scalar.activation(hi[:], ps[:], Act.Copy,
                                             bias=0.0, scale=s)
                        nc.vector.scalar_tensor_tensor(
                            lo[:], ps[:], s, hi[:],
                            op0=Alu.mult, op1=Alu.subtract)
                        qk_pair.append((hi, lo))
                    (q1, q2), (k1, k2) = qk_pair

                    # --- scores + grouped softmax per 128-query chunk ---
                    for qc in range(8):
                        sps = pss.tile([128, N], f32, tag="ss")
                        for half in range(2):
                            sl = ds(half * 512, 512)
                            passes = [(q1, k1), (q1, k2), (q2, k1)]
                            for pi, (qa, kb) in enumerate(passes):
                                nc.tensor.matmul(
                                    sps[:, sl], qa[:, ts(qc, 128)], kb[:, sl],
                                    start=(pi == 0), stop=(pi == 2))

                        gs = ep.tile([128, G], f32, tag="gs")
                        nc.vector.tensor_reduce(
                            gs[:], sps[:].rearrange("p (g j) -> p g j", j=GSIZE),
                            axis=AxX, op=Alu.max)
                        m1 = ep.tile([128, 1], f32, tag="m1")
                        nc.vector.tensor_reduce(m1[:], gs[:], axis=AxX, op=Alu.max)
                        eq = ep.tile([128, G], f32, tag="eq")
                        nc.vector.tensor_tensor(
                            eq[:], gs[:], m1[:].broadcast_to((128, G)),
                            op=Alu.is_ge)
                        gs2 = ep.tile([128, G], f32, tag="gs2")
                        nc.vector.scalar_tensor_tensor(
                            gs2[:], eq[:], -BIG, gs[:],
                            op0=Alu.mult, op1=Alu.add)
                        m2 = ep.tile([128, 1], f32, tag="m2")
                        nc.vector.tensor_reduce(m2[:], gs2[:], axis=AxX, op=Alu.max)
                        cmp = ep.tile([128, G], f32, tag="cmp")
                        nc.vector.tensor_tensor(
                            cmp[:], gs[:], m2[:].broadcast_to((128, G)),
                            op=Alu.is_ge)
                        m1b = ep.tile([128, 1], f32, tag="m1b")
                        nc.vector.tensor_reduce(m1b[:], gs[:], axis=AxX,
                                                op=Alu.max, negate=True)
                        # bias = cmp * (BIG + (-m1)) - BIG  (kept: -m1, masked: -BIG)
                        m1c = ep.tile([128, 1], f32, tag="m1c")
                        nc.vector.scalar_tensor_tensor(
                            m1c[:], m1b[:], BIG, m1b[:],
                            op0=Alu.add, op1=Alu.bypass)
                        bias = ep.tile([128, G], f32, tag="bias")
                        nc.vector.scalar_tensor_tensor(
                            bias[:], cmp[:], -BIG,
                            m1c[:].broadcast_to((128, G)),
                            op0=Alu.bypass, op1=Alu.mult)
                        nc.vector.tensor_scalar_add(bias[:], bias[:], -BIG)

                        masked = outp.tile([128, N], f32, tag="masked")
                        nc.vector.tensor_tensor(
                            masked[:].rearrange("p (g j) -> p g j", j=GSIZE),
                            sps[:].rearrange("p (g j) -> p g j", j=GSIZE),
                            bias[:].rearrange("p (g o) -> p g o", o=1)
                                .broadcast_to((128, G, GSIZE)),
                            op=Alu.add)
                        eo = outp.tile([128, N], f32, tag="eo")
                        rs = ep.tile([128, 1], f32, tag="rs")
                        nc.scalar.activation(eo[:], masked[:], Act.Exp,
                                             bias=0.0, scale=1.0,
                                             accum_out=rs[:])
                        rc = ep.tile([128, 1], f32, tag="rc")
                        nc.vector.reciprocal(rc[:], rs[:])
                        nc.scalar.activation(eo[:], eo[:], Act.Copy,
                                             bias=0.0, scale=rc[:])
                        nc.sync.dma_start(
                            out=out[ts(qc, 128), ds(h * N, N)], in_=eo[:])

    _install_legalizer(nc)
    return nc


_NC_CACHE = {}


def _get_nc():
    if "nc" not in _NC_CACHE:
        _NC_CACHE["nc"] = _build()
    return _NC_CACHE["nc"]


def _bf16_pair(a):
    hi = a.astype(ml_dtypes.bfloat16)
    lo = (a - hi.astype(np.float32)).astype(ml_dtypes.bfloat16)
    return hi, lo


def _in_maps(x, Wq, Wk):
    maps = []
    for c in range(NCORES):
        b, hh = c // 2, c % 2
        sl = slice(hh * HPC * DH, (hh + 1) * HPC * DH)
        xt1, xt2 = _bf16_pair(np.ascontiguousarray(x[b].T))
        wq1, wq2 = _bf16_pair(np.ascontiguousarray(Wq[:, sl]))
        wk1, wk2 = _bf16_pair(np.ascontiguousarray(Wk[:, sl]))
        maps.append({"xt1": xt1, "xt2": xt2, "wq1": wq1, "wq2": wq2,
                     "wk1": wk1, "wk2": wk2})
    return maps


def kernel(x, Wq, Wk, **kwargs):
    x = np.asarray(x, dtype=np.float32)
    Wq = np.asarray(Wq, dtype=np.float32)
    Wk = np.asarray(Wk, dtype=np.float32)
    nc = _get_nc()
    res = run_bass_kernel_spmd(nc, _in_maps(x, Wq, Wk),
                               core_ids=list(range(NCORES)))
    full = np.empty((B, N, H, N), dtype=np.float32)
    for c in range(NCORES):
        b, hh = c // 2, c % 2
        full[b, :, hh * HPC:(hh + 1) * HPC, :] = (
            res.results[c]["out"].reshape(N, HPC, N))
    return full



# revision 18
# speedup vs baseline: 1.1796x; 1.1796x over previous
"""GroupedRouter Bass kernel for 8 TRN2 NeuronCores.

Reference computation (per batch b, head h):
    q = x @ Wq, k = x @ Wk           (heads of dim 128)
    scores = q k^T / sqrt(128)       [N, N]
    group max over 8 key groups of 128, keep top-2 groups, softmax over kept.

Sharding: core c -> batch b = c//2, head half hh = c%2 (8 heads per core).
Fully data-parallel, no collectives.

Precision: every matmul = one fp16 main pass (fp16 products are exact in the
PE's fp22 pipeline) + one fp8(e4m3) DoubleRow correction pass at 0.5
cycles/row that carries the two first-order residual terms
(x2@W1 + x1@W2) with per-block power-of-2 scales chosen so all products land
at one PSUM scale. End-to-end score error ~6e-5, well inside the 2e-2 gate.
Projection residuals are prepared host-side; score-side residuals
(q2 = q - fp16(q)) are built on chip with a diag(-1) matmul into the proj
PSUM plus a second copyback.

Scale map (power-of-2, folded into copyback/exp scales):
  proj PSUM  = 2^15 * q     main: fp16(x*2^7) . fp16(W*2^8)
                            corr: e4(x2*2^10).e4(W1*2^5) + e4(x1*2^1).e4(W2*2^14)
  q1(fp16)   = 2^6  * q     copyback scale 2^-9
  qcat(e4m3) = [2^10 * q2 ; 2^2 * q1]   (copyback scale 2^-5; DVE scale 2^-4)
  score PSUM = 2^12 * s     main q1.k1, corr qcat.kcat (DoubleRow)
  exp        = exp(2^-12 * PSUM - 12)   (constant shift cancels in softmax)

Engines: PE matmuls ~210us; DVE group-max reduce + top-2 + per-group
mask/sum strips + normalize ~190us; ACT exp + sign ops + copybacks ~130us;
DMA fp16/fp8 in, bf16 out ~110us.
"""
import numpy as np
import orjson
import ml_dtypes

import concourse.bass as bass
import concourse.mybir as mybir
from concourse.tile import TileContext
from concourse.bass_utils import run_bass_kernel_spmd
from concourse.bass import ts, ds

B, N, D = 4, 1024, 2048
H, DH = 16, 128
G = 8
GSIZE = N // G          # 128
NCORES = 8
HPC = H // 2            # heads per core
SCALE = float(1.0 / np.sqrt(DH))

f32 = mybir.dt.float32
f16 = mybir.dt.float16
bf16 = mybir.dt.bfloat16
e4 = mybir.dt.float8e4
Alu = mybir.AluOpType
Act = mybir.ActivationFunctionType
AxX = mybir.AxisListType.X
DR = mybir.MatmulPerfMode.DoubleRow

# ---------------------------------------------------------------------------
# BIR sync-wait legalizer: walrus for cayman accepts only one sync-wait
# command per instruction; Tile attaches one per dependency. Hoist the excess
# onto standalone EventSemaphore instructions immediately before the target
# (engine queues are FIFO, so blocking semantics are unchanged).
# ---------------------------------------------------------------------------

def _legalize_bir(bir: dict) -> dict:
    ctr = 0
    for fn in bir["functions"]:
        for bb in fn["blocks"]:
            insts = bb.get("instructions")
            if not insts:
                continue
            out = []
            for ins in insts:
                si = ins.get("sync_info")
                waits = (si or {}).get("on_wait") or []
                if len(waits) > 1:
                    for w in waits[:-1]:
                        ctr += 1
                        out.append({
                            "engine": ins["engine"],
                            "ins": [],
                            "outs": [],
                            "name": f"legwait-{ctr}",
                            "opcode": "EventSemaphore",
                            "sync_info": {"on_update": [], "on_wait": [w]},
                        })
                    si["on_wait"] = waits[-1:]
                out.append(ins)
            bb["instructions"] = out
    return bir


def _install_legalizer(nc):
    orig = nc.to_json_bytes

    def to_json_bytes():
        return orjson.dumps(_legalize_bir(orjson.loads(orig())))

    nc.to_json_bytes = to_json_bytes


# ---------------------------------------------------------------------------
# Kernel build (one SPMD program; per-core differences live in the input data)
# ---------------------------------------------------------------------------

def _build():
    nc = bass.Bass()
    xm = nc.declare_dram_parameter("xm", [D, N], f16, isOutput=False)
    xc = nc.declare_dram_parameter("xc", [D, 2, N], e4, isOutput=False)
    wqm = nc.declare_dram_parameter("wqm", [D, HPC * DH], f16, isOutput=False)
    wkm = nc.declare_dram_parameter("wkm", [D, HPC * DH], f16, isOutput=False)
    wqc = nc.declare_dram_parameter("wqc", [D, 2, HPC * DH], e4, isOutput=False)
    wkc = nc.declare_dram_parameter("wkc", [D, 2, HPC * DH], e4, isOutput=False)
    ngi = nc.declare_dram_parameter("ngi", [128, 128], f16, isOutput=False)
    out = nc.declare_dram_parameter("out", [N, HPC * N], bf16, isOutput=True)

    nk = D // 128  # 16 contraction chunks
    xm3 = xm.rearrange("(kc p) t -> p kc t", p=128)
    xc4 = xc.rearrange("(kc p) two t -> p kc two t", p=128)
    wm3 = {"q": wqm.rearrange("(kc p) hd -> p kc hd", p=128),
           "k": wkm.rearrange("(kc p) hd -> p kc hd", p=128)}
    wc4 = {"q": wqc.rearrange("(kc p) two hd -> p kc two hd", p=128),
           "k": wkc.rearrange("(kc p) two hd -> p kc two hd", p=128)}

    with TileContext(nc) as tc, \
         nc.allow_low_precision("fp16/bf16 stages stay within the 2e-2 gate"):
        with tc.tile_pool(name="xT", bufs=1) as xtp, \
             tc.tile_pool(name="w", bufs=2) as wpool, \
             tc.tile_pool(name="qk", bufs=2) as qkp, \
             tc.tile_pool(name="psp", bufs=2, space="PSUM") as psp, \
             tc.tile_pool(name="pss", bufs=2, space="PSUM") as pss, \
             tc.tile_pool(name="ep", bufs=4) as ep, \
             tc.tile_pool(name="outp", bufs=3) as outp:
            cb = xtp.tile([128, 1], f32, name="cb", tag="cb")
            nc.vector.memset(cb[:], -12.0)
            negi = xtp.tile([128, 128], f16, name="negi", tag="negi")
            nc.sync.dma_start(out=negi[:], in_=ngi[:])

            # resident x: fp16 main [128, kc*t] and e4m3 residual pair
            # [128, kc*2*t]; chunked DMAs so matmuls start early. The W
            # residual pairs are also resident (their per-head 4D slices
            # can't be expressed as balanced DMA APs).
            xa = xtp.tile([128, nk * N], f16, name="xa", tag="xa")
            xb = xtp.tile([128, nk * 2 * N], e4, name="xb", tag="xb")
            XCH = 2
            for c in range(nk // XCH):
                sl = ds(c * XCH, XCH)
                nc.sync.dma_start(
                    out=xa[:, ds(c * XCH * N, XCH * N)].rearrange(
                        "p (kc t) -> p kc t", t=N),
                    in_=xm3[:, sl, :])
                nc.sync.dma_start(
                    out=xb[:, ds(c * XCH * 2 * N, XCH * 2 * N)].rearrange(
                        "p (kc two t) -> p kc two t", two=2, t=N),
                    in_=xc4[:, sl, :, :])
            HD = HPC * DH
            wcr = {}
            for mat, w4 in (("q", wc4["q"]), ("k", wc4["k"])):
                wt = xtp.tile([128, nk * 2 * HD], e4, name=f"wcr{mat}",
                              tag=f"wcr{mat}")
                for c in range(nk // XCH):
                    nc.sync.dma_start(
                        out=wt[:, ds(c * XCH * 2 * HD, XCH * 2 * HD)]
                        .rearrange("p (kc two hd) -> p kc two hd",
                                   two=2, hd=HD),
                        in_=w4[:, ds(c * XCH, XCH), :, :])
                wcr[mat] = wt

            def proj(h, mat):
                """Emit main+corr matmuls and the q1 copyback; return q1,
                qcat and a finalize closure (diag-subtract, residual
                copyback, e4m3 rescale). DoubleRow pairs element 0 with 0 and
                1 with 1, so the residual block goes to slot 0 on the q side
                and slot 1 on the k side."""
                res_blk = 0 if mat == "q" else 1
                wm = wpool.tile([128, nk * 128], f16, tag=f"wm{mat}")
                nc.sync.dma_start(
                    out=wm[:].rearrange("p (kc hd) -> p kc hd", hd=128),
                    in_=wm3[mat][:, :, ts(h, 128)])
                wc = wcr[mat]

                halves = []
                q1 = qkp.tile([128, N], f16, tag=f"q1{mat}")
                qcat = qkp.tile([128, 2 * N], e4, tag=f"qc{mat}")
                for half in range(2):
                    ps = psp.tile([128, 512], f32, tag="pp")
                    mv = ds(half * 512, 512)
                    for kc in range(nk):
                        nc.tensor.matmul(
                            ps[:], wm[:, ts(kc, 128)],
                            xa[:, ds(kc * N + half * 512, 512)],
                            start=(kc == 0), stop=False)
                    for kc in range(nk):
                        nc.tensor.matmul(
                            ps[:],
                            wc[:, ds(kc * 2 * HD, 2 * HD)].rearrange(
                                "p (two hd) -> p two hd", two=2)
                                [:, :, ts(h, 128)],
                            xb[:, ds(kc * 2 * N, 2 * N)].rearrange(
                                "p (two t) -> p two t", two=2)
                                [:, :, ds(half * 512, 512)],
                            start=False, stop=(kc == nk - 1),
                            perf_mode=DR, skip_group_check=True)
                    # q1 = 2^6 q (fp16)
                    nc.scalar.activation(q1[:, mv], ps[:], Act.Copy,
                                         bias=0.0, scale=2.0 ** -9)
                    halves.append(ps)

                def fin():
                    for half in range(2):
                        ps = halves[half]
                        mv = ds(half * 512, 512)
                        # ps -= 2^9 * q1  -> ps = 2^15 * (q - fp16(q))
                        nc.tensor.matmul(ps[:], negi[:], q1[:, mv],
                                         start=False, stop=True,
                                         skip_group_check=True)
                        # residual block: 2^10 * q2
                        nc.scalar.activation(
                            qcat[:, ds(res_blk * N + half * 512, 512)],
                            ps[:], Act.Copy, bias=0.0, scale=2.0 ** -5)
                    # main block: 2^2 * q-true = q1 * 2^-4
                    nc.vector.tensor_scalar(
                        qcat[:, ds((1 - res_blk) * N, N)], q1[:],
                        2.0 ** -4, None, op0=Alu.mult)

                return q1, qcat, fin

            def chain(h, qc, sps, eo16):
                # top-2 group selection from the score PSUM (scale-invariant)
                gs = ep.tile([128, G], f32, tag="gs")
                nc.vector.tensor_reduce(
                    gs[:], sps[:].rearrange("p (g j) -> p g j", j=GSIZE),
                    axis=AxX, op=Alu.max)
                m1en = ep.tile([128, 1], f32, tag="m1en")
                nc.vector.tensor_reduce(m1en[:], gs[:], axis=AxX,
                                        op=Alu.max, negate=True)
                sgn1 = ep.tile([128, G], f32, tag="sgn1")
                nc.scalar.activation(sgn1[:], gs[:], Act.Sign,
                                     bias=m1en[:], scale=1.0)
                gs2 = ep.tile([128, G], f32, tag="gs2")
                nc.vector.scalar_tensor_tensor(
                    gs2[:], sgn1[:], 0.0, gs[:], op0=Alu.is_lt, op1=Alu.mult)
                m2en = ep.tile([128, 1], f32, tag="m2en")
                nc.vector.tensor_reduce(m2en[:], gs2[:], axis=AxX,
                                        op=Alu.max, negate=True)
                sgn2 = ep.tile([128, G], f32, tag="sgn2")
                nc.scalar.activation(sgn2[:], gs[:], Act.Sign,
                                     bias=m2en[:], scale=1.0)
                cmp = ep.tile([128, G], f32, tag="cmp")
                nc.vector.tensor_scalar(cmp[:], sgn2[:], 0.0, None,
                                        op0=Alu.is_ge)
                # per-group strips: mask + per-group sums (4x bf16 DVE)
                eo2 = outp.tile([128, N], bf16, tag="eo2")
                gacc = ep.tile([128, G], f32, tag="gacc")
                for g in range(G):
                    nc.vector.tensor_scalar(
                        eo2[:, ts(g, GSIZE)], eo16[:, ts(g, GSIZE)],
                        cmp[:, ds(g, 1)], 0.0, op0=Alu.mult, op1=Alu.add,
                        accum_out=gacc[:, ds(g, 1)])
                ssum = ep.tile([128, 1], f32, tag="ssum")
                nc.vector.tensor_reduce(ssum[:], gacc[:], axis=AxX,
                                        op=Alu.add)
                rc = ep.tile([128, 1], f32, tag="rc")
                nc.vector.reciprocal(rc[:], ssum[:])
                eo3 = outp.tile([128, N], bf16, tag="eo3")
                nc.vector.tensor_scalar(eo3[:], eo2[:], rc[:], None,
                                        op0=Alu.mult)
                nc.sync.dma_start(
                    out=out[ts(qc, 128), ds(h * N, N)], in_=eo3[:])

            def scores(h, q1, qcat, k1, kcat):
                pend = None
                for qc in range(8):
                    sps = pss.tile([128, N], f32, tag="ss")
                    for half in range(2):
                        mv = ds(half * 512, 512)
                        nc.tensor.matmul(
                            sps[:, mv], q1[:, ts(qc, 128)], k1[:, mv],
                            start=True, stop=False)
                        nc.tensor.matmul(
                            sps[:, mv],
                            qcat[:].rearrange("p (two t) -> p two t", two=2)
                                [:, :, ts(qc, 128)],
                            kcat[:].rearrange("p (two t) -> p two t", two=2)
                                [:, :, ds(half * 512, 512)],
                            start=False, stop=True,
                            perf_mode=DR, skip_group_check=True)
                    eo16 = outp.tile([128, N], bf16, tag="eo16")
                    nc.scalar.activation(eo16[:], sps[:], Act.Exp,
                                         bias=cb[:], scale=2.0 ** -12)
                    if pend is not None:
                        chain(h, *pend)
                    pend = (qc, sps, eo16)
                chain(h, *pend)

            # software pipeline:
            #   [fin_k(h-1)] [Pq(h)] [scores(h-1)] [fin_q(h)] [Pk(h)]
            # so every proj-PSUM slot's final ops are emitted before the
            # ring slot is reallocated, and scores always have their
            # operands finalized a full proj block in advance.
            fin_k = None
            prev = None
            cur_q = None
            for h in range(HPC):
                if fin_k is not None:
                    fin_k()
                q1, qcat, fq = proj(h, "q")
                if prev is not None:
                    scores(h - 1, *prev)
                fq()
                k1, kcat, fk = proj(h, "k")
                fin_k = fk
                prev = (q1, qcat, k1, kcat)
            fin_k()
            scores(HPC - 1, *prev)

    _install_legalizer(nc)
    return nc


_NC_CACHE = {}


def _get_nc():
    if "nc" not in _NC_CACHE:
        _NC_CACHE["nc"] = _build()
    return _NC_CACHE["nc"]


_E4NP = mybir.dt.np(e4)


def _e4(a):
    return np.clip(np.asarray(a, np.float32), -240.0, 240.0).astype(_E4NP)


def _in_maps(x, Wq, Wk):
    negi = np.zeros((128, 128), np.float16)
    np.fill_diagonal(negi, -512.0)
    maps = []
    for c in range(NCORES):
        b, hh = c // 2, c % 2
        sl = slice(hh * HPC * DH, (hh + 1) * HPC * DH)
        xt = np.ascontiguousarray(x[b].T).astype(np.float32)
        xmain = (xt * 2.0 ** 7).astype(np.float16)
        x2 = xt - xmain.astype(np.float32) / 2.0 ** 7
        xcat = np.stack(
            [_e4(x2 * 2.0 ** 10), _e4(xt * 2.0 ** 1)], axis=1)
        m = {"xm": xmain, "xc": xcat, "ngi": negi}
        for nm, W in (("q", Wq[:, sl] * np.float32(SCALE)), ("k", Wk[:, sl])):
            W = np.ascontiguousarray(W).astype(np.float32)
            wmain = (W * 2.0 ** 8).astype(np.float16)
            w1 = wmain.astype(np.float32) / 2.0 ** 8
            w2 = W - w1
            wcat = np.stack([_e4(w1 * 2.0 ** 5), _e4(w2 * 2.0 ** 14)], axis=1)
            m[f"w{nm}m"] = wmain
            m[f"w{nm}c"] = wcat
        maps.append(m)
    return maps


def kernel(x, Wq, Wk, **kwargs):
    x = np.asarray(x, dtype=np.float32)
    Wq = np.asarray(Wq, dtype=np.float32)
    Wk = np.asarray(Wk, dtype=np.float32)
    nc = _get_nc()
    res = run_bass_kernel_spmd(nc, _in_maps(x, Wq, Wk),
                               core_ids=list(range(NCORES)))
    full = np.empty((B, N, H, N), dtype=np.float32)
    for c in range(NCORES):
        b, hh = c // 2, c % 2
        full[b, :, hh * HPC:(hh + 1) * HPC, :] = (
            res.results[c]["out"].astype(np.float32).reshape(N, HPC, N))
    return full


# revision 19
# speedup vs baseline: 1.2952x; 1.0980x over previous
"""GroupedRouter Bass kernel for 8 TRN2 NeuronCores.

Reference computation (per batch b, head h):
    q = x @ Wq, k = x @ Wk           (heads of dim 128)
    scores = q k^T / sqrt(128)       [N, N]
    group max over 8 key groups of 128, keep top-2 groups, softmax over kept.

Sharding: core c -> batch b = c//2, head half hh = c%2 (8 heads per core).
Fully data-parallel, no collectives.

Precision: every matmul = one fp16 main pass (fp16 products are exact in the
PE's fp22 pipeline) + one fp8(e4m3) DoubleRow correction pass at 0.5
cycles/row that carries the two first-order residual terms
(x2@W1 + x1@W2) with per-block power-of-2 scales chosen so all products land
at one PSUM scale. End-to-end score error ~6e-5, well inside the 2e-2 gate.
Projection residuals are prepared host-side; score-side residuals
(q2 = q - fp16(q)) are built on chip with a diag(-1) matmul into the proj
PSUM plus a second copyback.

Scale map (power-of-2, folded into copyback/exp scales):
  proj PSUM  = 2^15 * q     main: fp16(x*2^7) . fp16(W*2^8)
                            corr: e4(x2*2^10).e4(W1*2^5) + e4(x1*2^1).e4(W2*2^14)
  q1(fp16)   = 2^6  * q     copyback scale 2^-9
  qcat(e4m3) = [2^10 * q2 ; 2^2 * q1]   (copyback scale 2^-5; DVE scale 2^-4)
  score PSUM = 2^12 * s     main q1.k1, corr qcat.kcat (DoubleRow)
  exp        = exp(2^-12 * PSUM - 12)   (constant shift cancels in softmax)

Engines: PE matmuls ~210us; DVE group-max reduce + top-2 + per-group
mask/sum strips + normalize ~190us; ACT exp + sign ops + copybacks ~130us;
DMA fp16/fp8 in, bf16 out ~110us.
"""
import numpy as np
import orjson
import ml_dtypes

import concourse.bass as bass
import concourse.mybir as mybir
from concourse.tile import TileContext
from concourse.bass_utils import run_bass_kernel_spmd
from concourse.bass import ts, ds

B, N, D = 4, 1024, 2048
H, DH = 16, 128
G = 8
GSIZE = N // G          # 128
NCORES = 8
HPC = H // 2            # heads per core
SCALE = float(1.0 / np.sqrt(DH))

f32 = mybir.dt.float32
f16 = mybir.dt.float16
bf16 = mybir.dt.bfloat16
e4 = mybir.dt.float8e4
Alu = mybir.AluOpType
Act = mybir.ActivationFunctionType
AxX = mybir.AxisListType.X
DR = mybir.MatmulPerfMode.DoubleRow

# ---------------------------------------------------------------------------
# BIR sync-wait legalizer: walrus for cayman accepts only one sync-wait
# command per instruction; Tile attaches one per dependency. Hoist the excess
# onto standalone EventSemaphore instructions immediately before the target
# (engine queues are FIFO, so blocking semantics are unchanged).
# ---------------------------------------------------------------------------

def _legalize_bir(bir: dict) -> dict:
    ctr = 0
    for fn in bir["functions"]:
        for bb in fn["blocks"]:
            insts = bb.get("instructions")
            if not insts:
                continue
            out = []
            for ins in insts:
                si = ins.get("sync_info")
                waits = (si or {}).get("on_wait") or []
                if len(waits) > 1:
                    for w in waits[:-1]:
                        ctr += 1
                        out.append({
                            "engine": ins["engine"],
                            "ins": [],
                            "outs": [],
                            "name": f"legwait-{ctr}",
                            "opcode": "EventSemaphore",
                            "sync_info": {"on_update": [], "on_wait": [w]},
                        })
                    si["on_wait"] = waits[-1:]
                out.append(ins)
            bb["instructions"] = out
    return bir


def _install_legalizer(nc):
    orig = nc.to_json_bytes

    def to_json_bytes():
        return orjson.dumps(_legalize_bir(orjson.loads(orig())))

    nc.to_json_bytes = to_json_bytes


# ---------------------------------------------------------------------------
# Kernel build (one SPMD program; per-core differences live in the input data)
# ---------------------------------------------------------------------------

def _build():
    nc = bass.Bass()
    xm = nc.declare_dram_parameter("xm", [D, N], f16, isOutput=False)
    xc = nc.declare_dram_parameter("xc", [D, 2, N], e4, isOutput=False)
    wqm = nc.declare_dram_parameter("wqm", [D, HPC * DH], f16, isOutput=False)
    wkm = nc.declare_dram_parameter("wkm", [D, HPC * DH], f16, isOutput=False)
    wqc = nc.declare_dram_parameter("wqc", [D, 2, HPC * DH], e4, isOutput=False)
    wkc = nc.declare_dram_parameter("wkc", [D, 2, HPC * DH], e4, isOutput=False)
    ngi = nc.declare_dram_parameter("ngi", [128, 128], f16, isOutput=False)
    out = nc.declare_dram_parameter("out", [N, HPC * N], bf16, isOutput=True)

    nk = D // 128  # 16 contraction chunks
    xm3 = xm.rearrange("(kc p) t -> p kc t", p=128)
    xc4 = xc.rearrange("(kc p) two t -> p kc two t", p=128)
    wm3 = {"q": wqm.rearrange("(kc p) hd -> p kc hd", p=128),
           "k": wkm.rearrange("(kc p) hd -> p kc hd", p=128)}
    wc4 = {"q": wqc.rearrange("(kc p) two hd -> p kc two hd", p=128),
           "k": wkc.rearrange("(kc p) two hd -> p kc two hd", p=128)}

    with TileContext(nc) as tc, \
         nc.allow_low_precision("fp16/bf16 stages stay within the 2e-2 gate"):
        with tc.tile_pool(name="xT", bufs=1) as xtp, \
             tc.tile_pool(name="w", bufs=2) as wpool, \
             tc.tile_pool(name="qk", bufs=2) as qkp, \
             tc.tile_pool(name="psp", bufs=4, space="PSUM") as psp, \
             tc.tile_pool(name="pss", bufs=2, space="PSUM") as pss, \
             tc.tile_pool(name="ep", bufs=4) as ep, \
             tc.tile_pool(name="outp", bufs=3) as outp:
            cb = xtp.tile([128, 1], f32, name="cb", tag="cb")
            nc.vector.memset(cb[:], -12.0)
            negi = xtp.tile([128, 128], f16, name="negi", tag="negi")
            nc.sync.dma_start(out=negi[:], in_=ngi[:])

            # resident x: fp16 main [128, kc*t] and e4m3 residual pair
            # [128, kc*2*t]; chunked DMAs so matmuls start early. The W
            # residual pairs are also resident (their per-head 4D slices
            # can't be expressed as balanced DMA APs).
            xa = xtp.tile([128, nk * N], f16, name="xa", tag="xa")
            xb = xtp.tile([128, nk * 2 * N], e4, name="xb", tag="xb")
            XCH = 2
            for c in range(nk // XCH):
                sl = ds(c * XCH, XCH)
                nc.sync.dma_start(
                    out=xa[:, ds(c * XCH * N, XCH * N)].rearrange(
                        "p (kc t) -> p kc t", t=N),
                    in_=xm3[:, sl, :])
                nc.sync.dma_start(
                    out=xb[:, ds(c * XCH * 2 * N, XCH * 2 * N)].rearrange(
                        "p (kc two t) -> p kc two t", two=2, t=N),
                    in_=xc4[:, sl, :, :])
            HD = HPC * DH
            wcr = {}
            for mat, w4 in (("q", wc4["q"]), ("k", wc4["k"])):
                wt = xtp.tile([128, nk * 2 * HD], e4, name=f"wcr{mat}",
                              tag=f"wcr{mat}")
                for c in range(nk // XCH):
                    nc.sync.dma_start(
                        out=wt[:, ds(c * XCH * 2 * HD, XCH * 2 * HD)]
                        .rearrange("p (kc two hd) -> p kc two hd",
                                   two=2, hd=HD),
                        in_=w4[:, ds(c * XCH, XCH), :, :])
                wcr[mat] = wt

            def proj(h, mat):
                """Emit main+corr matmuls and the q1 copyback; return q1,
                qcat and a finalize closure (diag-subtract, residual
                copyback, e4m3 rescale). DoubleRow pairs element 0 with 0 and
                1 with 1, so the residual block goes to slot 0 on the q side
                and slot 1 on the k side."""
                res_blk = 0 if mat == "q" else 1
                wm = wpool.tile([128, nk * 128], f16, tag=f"wm{mat}")
                nc.sync.dma_start(
                    out=wm[:].rearrange("p (kc hd) -> p kc hd", hd=128),
                    in_=wm3[mat][:, :, ts(h, 128)])
                wc = wcr[mat]

                halves = []
                q1 = qkp.tile([128, N], f16, tag=f"q1{mat}")
                qcat = qkp.tile([128, 2 * N], e4, tag=f"qc{mat}")
                for half in range(2):
                    ps = psp.tile([128, 512], f32, tag="pp")
                    mv = ds(half * 512, 512)
                    for kc in range(nk):
                        nc.tensor.matmul(
                            ps[:], wm[:, ts(kc, 128)],
                            xa[:, ds(kc * N + half * 512, 512)],
                            start=(kc == 0), stop=False)
                    for kc in range(nk):
                        nc.tensor.matmul(
                            ps[:],
                            wc[:, ds(kc * 2 * HD, 2 * HD)].rearrange(
                                "p (two hd) -> p two hd", two=2)
                                [:, :, ts(h, 128)],
                            xb[:, ds(kc * 2 * N, 2 * N)].rearrange(
                                "p (two t) -> p two t", two=2)
                                [:, :, ds(half * 512, 512)],
                            start=False, stop=(kc == nk - 1),
                            perf_mode=DR, skip_group_check=True)
                    # q1 = 2^6 q (fp16)
                    nc.scalar.activation(q1[:, mv], ps[:], Act.Copy,
                                         bias=0.0, scale=2.0 ** -9)
                    halves.append(ps)

                def fin():
                    for half in range(2):
                        ps = halves[half]
                        mv = ds(half * 512, 512)
                        # ps -= 2^9 * q1  -> ps = 2^15 * (q - fp16(q))
                        nc.tensor.matmul(ps[:], negi[:], q1[:, mv],
                                         start=False, stop=True,
                                         skip_group_check=True)
                        # residual block: 2^10 * q2
                        nc.scalar.activation(
                            qcat[:, ds(res_blk * N + half * 512, 512)],
                            ps[:], Act.Copy, bias=0.0, scale=2.0 ** -5)
                    # main block: 2^2 * q-true = q1 * 2^-4
                    nc.vector.tensor_scalar(
                        qcat[:, ds((1 - res_blk) * N, N)], q1[:],
                        2.0 ** -4, None, op0=Alu.mult)

                return q1, qcat, fin

            def chain(h, qc, sps, eo16):
                # top-2 group selection from the score PSUM (scale-invariant)
                gs = ep.tile([128, G], f32, tag="gs")
                nc.vector.tensor_reduce(
                    gs[:], sps[:].rearrange("p (g j) -> p g j", j=GSIZE),
                    axis=AxX, op=Alu.max)
                m1en = ep.tile([128, 1], f32, tag="m1en")
                nc.vector.tensor_reduce(m1en[:], gs[:], axis=AxX,
                                        op=Alu.max, negate=True)
                sgn1 = ep.tile([128, G], f32, tag="sgn1")
                nc.scalar.activation(sgn1[:], gs[:], Act.Sign,
                                     bias=m1en[:], scale=1.0)
                gs2 = ep.tile([128, G], f32, tag="gs2")
                nc.vector.scalar_tensor_tensor(
                    gs2[:], sgn1[:], 0.0, gs[:], op0=Alu.is_lt, op1=Alu.mult)
                m2en = ep.tile([128, 1], f32, tag="m2en")
                nc.vector.tensor_reduce(m2en[:], gs2[:], axis=AxX,
                                        op=Alu.max, negate=True)
                sgn2 = ep.tile([128, G], f32, tag="sgn2")
                nc.scalar.activation(sgn2[:], gs[:], Act.Sign,
                                     bias=m2en[:], scale=1.0)
                cmp = ep.tile([128, G], f32, tag="cmp")
                nc.vector.tensor_scalar(cmp[:], sgn2[:], 0.0, None,
                                        op0=Alu.is_ge)
                # per-group strips: mask + per-group sums (4x bf16 DVE)
                eo2 = outp.tile([128, N], bf16, tag="eo2")
                gacc = ep.tile([128, G], f32, tag="gacc")
                for g in range(G):
                    nc.vector.tensor_scalar(
                        eo2[:, ts(g, GSIZE)], eo16[:, ts(g, GSIZE)],
                        cmp[:, ds(g, 1)], 0.0, op0=Alu.mult, op1=Alu.add,
                        accum_out=gacc[:, ds(g, 1)])
                ssum = ep.tile([128, 1], f32, tag="ssum")
                nc.vector.tensor_reduce(ssum[:], gacc[:], axis=AxX,
                                        op=Alu.add)
                rc = ep.tile([128, 1], f32, tag="rc")
                nc.vector.reciprocal(rc[:], ssum[:])
                eo3 = outp.tile([128, N], bf16, tag="eo3")
                nc.vector.tensor_scalar(eo3[:], eo2[:], rc[:], None,
                                        op0=Alu.mult)
                nc.sync.dma_start(
                    out=out[ts(qc, 128), ds(h * N, N)], in_=eo3[:])

            def scores(h, q1, qcat, k1, kcat):
                pend = None
                for qc in range(8):
                    sps = pss.tile([128, N], f32, tag="ss")
                    for half in range(2):
                        mv = ds(half * 512, 512)
                        nc.tensor.matmul(
                            sps[:, mv], q1[:, ts(qc, 128)], k1[:, mv],
                            start=True, stop=False)
                        nc.tensor.matmul(
                            sps[:, mv],
                            qcat[:].rearrange("p (two t) -> p two t", two=2)
                                [:, :, ts(qc, 128)],
                            kcat[:].rearrange("p (two t) -> p two t", two=2)
                                [:, :, ds(half * 512, 512)],
                            start=False, stop=True,
                            perf_mode=DR, skip_group_check=True)
                    eo16 = outp.tile([128, N], bf16, tag="eo16")
                    nc.scalar.activation(eo16[:], sps[:], Act.Exp,
                                         bias=cb[:], scale=2.0 ** -12)
                    if pend is not None:
                        chain(h, *pend)
                    pend = (qc, sps, eo16)
                chain(h, *pend)

            # software pipeline:
            #   [fin_k(h-1)] [Pq(h)] [scores(h-1)] [fin_q(h)] [Pk(h)]
            # so every proj-PSUM slot's final ops are emitted before the
            # ring slot is reallocated, and scores always have their
            # operands finalized a full proj block in advance.
            fin_k = None
            prev = None
            cur_q = None
            for h in range(HPC):
                if fin_k is not None:
                    fin_k()
                q1, qcat, fq = proj(h, "q")
                if prev is not None:
                    scores(h - 1, *prev)
                fq()
                k1, kcat, fk = proj(h, "k")
                fin_k = fk
                prev = (q1, qcat, k1, kcat)
            fin_k()
            scores(HPC - 1, *prev)

    _install_legalizer(nc)
    return nc


_NC_CACHE = {}


def _get_nc():
    if "nc" not in _NC_CACHE:
        _NC_CACHE["nc"] = _build()
    return _NC_CACHE["nc"]


_E4NP = mybir.dt.np(e4)


def _e4(a):
    return np.clip(np.asarray(a, np.float32), -240.0, 240.0).astype(_E4NP)


def _in_maps(x, Wq, Wk):
    negi = np.zeros((128, 128), np.float16)
    np.fill_diagonal(negi, -512.0)
    maps = []
    for c in range(NCORES):
        b, hh = c // 2, c % 2
        sl = slice(hh * HPC * DH, (hh + 1) * HPC * DH)
        xt = np.ascontiguousarray(x[b].T).astype(np.float32)
        xmain = (xt * 2.0 ** 7).astype(np.float16)
        x2 = xt - xmain.astype(np.float32) / 2.0 ** 7
        xcat = np.stack(
            [_e4(x2 * 2.0 ** 10), _e4(xt * 2.0 ** 1)], axis=1)
        m = {"xm": xmain, "xc": xcat, "ngi": negi}
        for nm, W in (("q", Wq[:, sl] * np.float32(SCALE)), ("k", Wk[:, sl])):
            W = np.ascontiguousarray(W).astype(np.float32)
            wmain = (W * 2.0 ** 8).astype(np.float16)
            w1 = wmain.astype(np.float32) / 2.0 ** 8
            w2 = W - w1
            wcat = np.stack([_e4(w1 * 2.0 ** 5), _e4(w2 * 2.0 ** 14)], axis=1)
            m[f"w{nm}m"] = wmain
            m[f"w{nm}c"] = wcat
        maps.append(m)
    return maps


def kernel(x, Wq, Wk, **kwargs):
    x = np.asarray(x, dtype=np.float32)
    Wq = np.asarray(Wq, dtype=np.float32)
    Wk = np.asarray(Wk, dtype=np.float32)
    nc = _get_nc()
    res = run_bass_kernel_spmd(nc, _in_maps(x, Wq, Wk),
                               core_ids=list(range(NCORES)))
    full = np.empty((B, N, H, N), dtype=np.float32)
    for c in range(NCORES):
        b, hh = c // 2, c % 2
        full[b, :, hh * HPC:(hh + 1) * HPC, :] = (
            res.results[c]["out"].astype(np.float32).reshape(N, HPC, N))
    return full
